# revision 3
# baseline (speedup 1.0000x reference)
"""Grouped MoE MLP (SwiGLU) kernel for Trainium2, 8 NeuronCores.

Strategy (pure expert-parallel):
  Tokens arrive pre-sorted by expert with per-expert counts.  Expert e's
  weights and token block go to core e (one expert per core, no weight
  duplication -- weight bytes shipped to the device are the dominant cost
  for this problem).  Token blocks are zero-padded to a common T_pad so
  all cores run one SPMD program.

  Device program per core (dense SwiGLU MLP over T_pad tokens):
    GEMM1: h1^T[f, t] = sum_h W1[h, f] * x[t, h]     (h on partitions)
    SwiGLU on feature-partitioned tiles
    GEMM2: out[t, o]  = sum_f h[t, f] * W2[f, o]     (f on partitions,
           tokens become the PSUM partition dim so the output lands in
           natural [T, HIDDEN] layout -- no transposes on the way out)

  All device I/O is bf16 (inputs cast on host, output cast back to f32
  on host) to halve the bytes staged over the host<->device link.
"""

import math
from contextlib import ExitStack

import ml_dtypes
import numpy as np

P = 128
HIDDEN = 2048
INTER = 1408
GU = 2 * INTER            # 2816 = gate+up columns
KH = HIDDEN // P          # 16 k-tiles for GEMM1
KI = INTER // P           # 11 k-tiles for GEMM2 / gate-up pair blocks
NO = HIDDEN // 512        # 4 output column blocks of 512
N_CORES = 8
NT = 512                  # tokens per chunk (matmul moving free dim)

BF16 = ml_dtypes.bfloat16

_PROGRAM_CACHE: dict = {}


def _chunks(rows: int, nt: int):
    out = []
    r = 0
    while r < rows:
        c = min(nt, rows - r)
        out.append((r, c))
        r += c
    return out


def _build_program(t_pad: int, nt: int):
    import concourse.mybir as mybir
    import concourse.tile as tile
    from concourse import bacc

    bf16 = mybir.dt.bfloat16
    f32 = mybir.dt.float32

    nc = bacc.Bacc(None, target_bir_lowering=False, debug=False)
    xT = nc.dram_tensor("xT", [KH, P, t_pad], bf16, kind="ExternalInput")
    w1 = nc.dram_tensor("w1", [KH, P, GU], bf16, kind="ExternalInput")
    w2 = nc.dram_tensor("w2", [KI, P, HIDDEN], bf16, kind="ExternalInput")
    out = nc.dram_tensor("out", [t_pad, HIDDEN], bf16, kind="ExternalOutput")

    with tile.TileContext(nc) as tc, ExitStack() as ctx:
        w1_pool = ctx.enter_context(tc.tile_pool(name="w1p", bufs=1))
        w2_pool = ctx.enter_context(tc.tile_pool(name="w2p", bufs=1))
        x_pool = ctx.enter_context(tc.tile_pool(name="xp", bufs=2))
        h_pool = ctx.enter_context(tc.tile_pool(name="hp", bufs=2))
        g_pool = ctx.enter_context(tc.tile_pool(name="gp", bufs=2))
        o_pool = ctx.enter_context(tc.tile_pool(name="op", bufs=3))
        ps1 = ctx.enter_context(tc.tile_pool(name="ps1", bufs=2, space="PSUM"))
        ps2 = ctx.enter_context(tc.tile_pool(name="ps2", bufs=2, space="PSUM"))

        chunk_list = _chunks(t_pad, nt)
        # first x chunk goes ahead of the weight DMAs so the first GEMM1
        # matmul only waits for w1, not the whole weight set (HWDGE
        # queues are FIFO)
        c0_off, c0_n = chunk_list[0]
        xt0 = x_pool.tile([P, KH, c0_n], bf16, tag="xt")
        for k in range(KH):
            nc.sync.dma_start(xt0[:, k, :], xT[k, :, c0_off : c0_off + c0_n])
        w1t = w1_pool.tile([P, KH, GU], bf16)
        for k in range(KH):
            nc.sync.dma_start(w1t[:, k, :], w1[k])
        w2t = w2_pool.tile([P, KI, HIDDEN], bf16)
        for k in range(KI):
            nc.sync.dma_start(w2t[:, k, :], w2[k])

        for ci, (t0, c_n) in enumerate(chunk_list):
            if ci == 0:
                xt = xt0
            else:
                xt = x_pool.tile([P, KH, c_n], bf16, tag="xt")
                for k in range(KH):
                    nc.sync.dma_start(xt[:, k, :], xT[k, :, t0 : t0 + c_n])
            ht = h_pool.tile([P, KI, c_n], bf16, tag="ht")
            for mp in range(KI):
                pg = ps1.tile([P, c_n], f32, tag="pg")
                pu = ps1.tile([P, c_n], f32, tag="pu")
                for k in range(KH):
                    nc.tensor.matmul(
                        pg[:],
                        w1t[:, k, mp * P : (mp + 1) * P],
                        xt[:, k, :],
                        start=(k == 0),
                        stop=(k == KH - 1),
                    )
                for k in range(KH):
                    nc.tensor.matmul(
                        pu[:],
                        w1t[:, k, (KI + mp) * P : (KI + mp + 1) * P],
                        xt[:, k, :],
                        start=(k == 0),
                        stop=(k == KH - 1),
                    )
                gt = g_pool.tile([P, c_n], bf16, tag="gt")
                nc.scalar.activation(
                    gt[:], pg[:], mybir.ActivationFunctionType.Silu
                )
                nc.vector.tensor_mul(ht[:, mp, :], gt[:], pu[:])
            # GEMM2 with tokens on the PSUM partition dim: for each
            # 128-token block, out[tb, o_blk] = ht[:, :, tb].T @ w2
            for tb in range(c_n // P):
                for m in range(NO):
                    po = ps2.tile([P, 512], f32, tag="po")
                    for k in range(KI):
                        nc.tensor.matmul(
                            po[:],
                            ht[:, k, tb * P : (tb + 1) * P],
                            w2t[:, k, m * 512 : (m + 1) * 512],
                            start=(k == 0),
                            stop=(k == KI - 1),
                        )
                    om = o_pool.tile([P, 512], bf16, tag="om")
                    nc.vector.tensor_copy(om[:], po[:])
                    nc.sync.dma_start(
                        out[t0 + tb * P : t0 + (tb + 1) * P, m * 512 : (m + 1) * 512],
                        om[:],
                    )
    nc.compile()
    return nc


def _get_program(t_pad: int, nt: int):
    key = (t_pad, nt)
    if key not in _PROGRAM_CACHE:
        _PROGRAM_CACHE[key] = _build_program(t_pad, nt)
    return _PROGRAM_CACHE[key]


def _pack_w1(w: np.ndarray) -> np.ndarray:
    # [HIDDEN, GU] f32 -> [KH, P, GU] bf16 (pure reshape + cast)
    return w.reshape(KH, P, GU).astype(BF16)


def _pack_w2(w: np.ndarray) -> np.ndarray:
    # [INTER, HIDDEN] f32 -> [KI, P, HIDDEN] bf16 (pure reshape + cast)
    return w.reshape(KI, P, HIDDEN).astype(BF16)


def _assign_experts(counts):
    """Expert e -> core (e % N_CORES); slots stack when E > N_CORES."""
    n_exp = len(counts)
    n_slots = max(1, math.ceil(n_exp / N_CORES))
    cores = []
    for r in range(N_CORES):
        slots = []
        for s in range(n_slots):
            e = s * N_CORES + r
            slots.append(e if e < n_exp else None)
        cores.append(slots)
    return cores, n_slots


def _run(
    hidden_states: np.ndarray,
    merged_gate_up_proj: np.ndarray,
    merged_down_proj: np.ndarray,
    num_tokens_per_expert: np.ndarray,
    trace: bool = False,
):
    counts = [int(c) for c in np.asarray(num_tokens_per_expert)]
    n_exp = len(counts)
    offs = np.concatenate([[0], np.cumsum(counts)]).astype(int)
    total = int(offs[-1])

    core_experts, n_slots = _assign_experts(counts)
    assert n_slots == 1, "more experts than cores not supported"
    per_core_rows = [sum(counts[e] for e in slots if e is not None)
                     for slots in core_experts]
    t_pad = max(NT, ((max(per_core_rows) + NT - 1) // NT) * NT)

    nc = _get_program(t_pad, NT)

    # [TOTAL, HIDDEN] f32 -> bf16 -> transposed [HIDDEN, TOTAL] -> [KH, P, TOTAL]
    xT_full = np.ascontiguousarray(hidden_states.astype(BF16).T).reshape(
        KH, P, total
    )

    in_maps = []
    for r in range(N_CORES):
        e = core_experts[r][0]
        if e is None or counts[e] == 0:
            e_w = 0 if n_exp else None
            xT_core = np.zeros((KH, P, t_pad), dtype=BF16)
            in_maps.append(
                {
                    "xT": xT_core,
                    "w1": _pack_w1(merged_gate_up_proj[e_w]),
                    "w2": _pack_w2(merged_down_proj[e_w]),
                }
            )
            continue
        cnt = counts[e]
        xT_core = np.zeros((KH, P, t_pad), dtype=BF16)
        xT_core[:, :, :cnt] = xT_full[:, :, offs[e] : offs[e] + cnt]
        in_maps.append(
            {
                "xT": xT_core,
                "w1": _pack_w1(merged_gate_up_proj[e]),
                "w2": _pack_w2(merged_down_proj[e]),
            }
        )

    res = _execute(nc, in_maps, trace)

    out = np.empty((total, HIDDEN), dtype=np.float32)
    for r in range(N_CORES):
        e = core_experts[r][0]
        if e is None or counts[e] == 0:
            continue
        cnt = counts[e]
        o_core = res.results[r]["out"]
        out[offs[e] : offs[e] + cnt] = o_core[:cnt].astype(np.float32)
    return out, res


def _execute(nc, in_maps, trace):
    from concourse.bass_utils import run_bass_kernel_spmd

    if not trace:
        try:
            return _execute_pjrt_dev_zeros(nc, in_maps)
        except Exception:
            pass
    return run_bass_kernel_spmd(nc, in_maps, list(range(N_CORES)), trace=trace)


def _execute_pjrt_dev_zeros(nc, in_maps):
    """run_bass_via_pjrt equivalent, but the donated zero output buffers
    are created on-device (jnp.zeros under jit) instead of being staged
    from host numpy -- saves shipping one full output-sized array of
    zeros per core over the host->device link."""
    from concourse.bass_utils import BassKernelResults, axon_active
    import concourse.mybir as mybir
    from concourse import bass2jax
    import jax
    import jax.numpy as jnp
    from jax.sharding import Mesh, PartitionSpec, NamedSharding
    from jax.experimental.shard_map import shard_map

    if not axon_active():
        raise RuntimeError("pjrt path requires axon")
    if nc.dbg_addr is not None:
        raise RuntimeError("debug program")

    bass2jax.install_neuronx_cc_hook()

    partition_name = nc.partition_id_tensor.name if nc.partition_id_tensor else None
    in_names, out_names, out_avals = [], [], []
    for alloc in nc.m.functions[0].allocations:
        if not isinstance(alloc, mybir.MemoryLocationSet):
            continue
        name = alloc.memorylocations[0].name
        if alloc.kind == "ExternalInput":
            if name != partition_name:
                in_names.append(name)
        elif alloc.kind == "ExternalOutput":
            out_names.append(name)
            out_avals.append(
                jax.core.ShapedArray(
                    tuple(alloc.tensor_shape), mybir.dt.np(alloc.dtype)
                )
            )
    n_params = len(in_names)
    n_outs = len(out_avals)
    all_names = in_names + out_names
    if partition_name is not None:
        all_names = all_names + [partition_name]
    donate = tuple(range(n_params, n_params + n_outs))

    def _body(*args):
        operands = list(args)
        if partition_name is not None:
            operands.append(bass2jax.partition_id_tensor())
        outs = bass2jax._bass_exec_p.bind(
            *operands,
            out_avals=tuple(out_avals),
            in_names=tuple(all_names),
            out_names=tuple(out_names),
            lowering_input_output_aliases=(),
            sim_require_finite=True,
            sim_require_nnan=True,
            nc=nc,
        )
        return tuple(outs)

    devices = jax.devices()[:N_CORES]
    assert len(devices) == N_CORES
    mesh = Mesh(np.asarray(devices), ("core",))
    in_specs = (PartitionSpec("core"),) * (n_params + n_outs)
    out_specs = (PartitionSpec("core"),) * n_outs
    sharded = jax.jit(
        shard_map(
            _body, mesh=mesh, in_specs=in_specs, out_specs=out_specs,
            check_rep=False,
        ),
        donate_argnums=donate,
        keep_unused=True,
    )
    concat_in = [
        np.concatenate([np.asarray(m[name]) for m in in_maps], axis=0)
        for name in in_names
    ]
    zsharding = NamedSharding(mesh, PartitionSpec("core"))
    dev_zeros = [
        jax.jit(
            lambda s=av.shape, d=av.dtype: jnp.zeros(
                (N_CORES * s[0], *s[1:]), d
            ),
            out_shardings=zsharding,
        )()
        for av in out_avals
    ]
    out_arrs = sharded(*concat_in, *dev_zeros)
    results = [
        {
            name: np.asarray(out_arrs[i]).reshape(
                N_CORES, *out_avals[i].shape
            )[c]
            for i, name in enumerate(out_names)
        }
        for c in range(N_CORES)
    ]
    return BassKernelResults(
        results=results,
        instructions_and_trace=None,
        profile_json=None,
        exec_time_ns=None,
    )


def kernel(**inputs) -> np.ndarray:
    return _run(**inputs, trace=False)[0]


def run_traced(**inputs):
    return _run(**inputs, trace=True)


# revision 6
# speedup vs baseline: 1.0098x; 1.0098x over previous
"""Grouped MoE MLP (SwiGLU) kernel for Trainium2, 8 NeuronCores.

Strategy (pure expert-parallel):
  Tokens arrive pre-sorted by expert with per-expert counts.  Expert e's
  weights and token block go to core e (one expert per core, no weight
  duplication -- weight bytes shipped to the device are the dominant cost
  for this problem).  Token blocks are zero-padded to a common T_pad so
  all cores run one SPMD program.

  Device program per core (dense SwiGLU MLP over T_pad tokens):
    GEMM1: h1^T[f, t] = sum_h W1[h, f] * x[t, h]     (h on partitions)
    SwiGLU on feature-partitioned tiles
    GEMM2: out[t, o]  = sum_f h[t, f] * W2[f, o]     (f on partitions,
           tokens become the PSUM partition dim so the output lands in
           natural [T, HIDDEN] layout -- no transposes on the way out)

  All device I/O is bf16 (inputs cast on host, output cast back to f32
  on host) to halve the bytes staged over the host<->device link.
"""

import math
from contextlib import ExitStack

import ml_dtypes
import numpy as np

P = 128
HIDDEN = 2048
INTER = 1408
GU = 2 * INTER            # 2816 = gate+up columns
KH = HIDDEN // P          # 16 k-tiles for GEMM1
KI = INTER // P           # 11 k-tiles for GEMM2 / gate-up pair blocks
NO = HIDDEN // 512        # 4 output column blocks of 512
N_CORES = 8
NT = 512                  # tokens per chunk (matmul moving free dim)

BF16 = ml_dtypes.bfloat16

_PROGRAM_CACHE: dict = {}


def _chunks(rows: int, nt: int):
    out = []
    r = 0
    while r < rows:
        c = min(nt, rows - r)
        out.append((r, c))
        r += c
    return out


def _build_program(t_pad: int, nt: int):
    import concourse.mybir as mybir
    import concourse.tile as tile
    from concourse import bacc

    bf16 = mybir.dt.bfloat16
    f32 = mybir.dt.float32

    nc = bacc.Bacc(None, target_bir_lowering=False, debug=False)
    xT = nc.dram_tensor("xT", [KH, P, t_pad], bf16, kind="ExternalInput")
    w1 = nc.dram_tensor("w1", [KH, P, GU], bf16, kind="ExternalInput")
    w2 = nc.dram_tensor("w2", [KI, P, HIDDEN], bf16, kind="ExternalInput")
    out = nc.dram_tensor("out", [t_pad, HIDDEN], bf16, kind="ExternalOutput")

    with tile.TileContext(nc) as tc, ExitStack() as ctx:
        w1_pool = ctx.enter_context(tc.tile_pool(name="w1p", bufs=1))
        w2_pool = ctx.enter_context(tc.tile_pool(name="w2p", bufs=1))
        x_pool = ctx.enter_context(tc.tile_pool(name="xp", bufs=2))
        h_pool = ctx.enter_context(tc.tile_pool(name="hp", bufs=2))
        g_pool = ctx.enter_context(tc.tile_pool(name="gp", bufs=2))
        o_pool = ctx.enter_context(tc.tile_pool(name="op", bufs=3))
        ps1 = ctx.enter_context(tc.tile_pool(name="ps1", bufs=2, space="PSUM"))
        ps2 = ctx.enter_context(tc.tile_pool(name="ps2", bufs=2, space="PSUM"))

        chunk_list = _chunks(t_pad, nt)
        # first x chunk goes ahead of the weight DMAs so the first GEMM1
        # matmul only waits for w1, not the whole weight set (HWDGE
        # queues are FIFO)
        c0_off, c0_n = chunk_list[0]
        xt0 = x_pool.tile([P, KH, c0_n], bf16, tag="xt")
        for k in range(KH):
            nc.sync.dma_start(xt0[:, k, :], xT[k, :, c0_off : c0_off + c0_n])
        w1t = w1_pool.tile([P, KH, GU], bf16)
        for k in range(KH):
            nc.sync.dma_start(w1t[:, k, :], w1[k])
        w2t = w2_pool.tile([P, KI, HIDDEN], bf16)
        for k in range(KI):
            nc.sync.dma_start(w2t[:, k, :], w2[k])

        for ci, (t0, c_n) in enumerate(chunk_list):
            if ci == 0:
                xt = xt0
            else:
                xt = x_pool.tile([P, KH, c_n], bf16, tag="xt")
                for k in range(KH):
                    nc.sync.dma_start(xt[:, k, :], xT[k, :, t0 : t0 + c_n])
            ht = h_pool.tile([P, KI, c_n], bf16, tag="ht")
            for mp in range(KI):
                pg = ps1.tile([P, c_n], f32, tag="pg")
                pu = ps1.tile([P, c_n], f32, tag="pu")
                for k in range(KH):
                    nc.tensor.matmul(
                        pg[:],
                        w1t[:, k, mp * P : (mp + 1) * P],
                        xt[:, k, :],
                        start=(k == 0),
                        stop=(k == KH - 1),
                    )
                for k in range(KH):
                    nc.tensor.matmul(
                        pu[:],
                        w1t[:, k, (KI + mp) * P : (KI + mp + 1) * P],
                        xt[:, k, :],
                        start=(k == 0),
                        stop=(k == KH - 1),
                    )
                gt = g_pool.tile([P, c_n], bf16, tag="gt")
                nc.scalar.activation(
                    gt[:], pg[:], mybir.ActivationFunctionType.Silu
                )
                nc.vector.tensor_mul(ht[:, mp, :], gt[:], pu[:])
            # GEMM2 with tokens on the PSUM partition dim: for each
            # 128-token block, out[tb, o_blk] = ht[:, :, tb].T @ w2
            for tb in range(c_n // P):
                for m in range(NO):
                    po = ps2.tile([P, 512], f32, tag="po")
                    for k in range(KI):
                        nc.tensor.matmul(
                            po[:],
                            ht[:, k, tb * P : (tb + 1) * P],
                            w2t[:, k, m * 512 : (m + 1) * 512],
                            start=(k == 0),
                            stop=(k == KI - 1),
                        )
                    om = o_pool.tile([P, 512], bf16, tag="om")
                    nc.vector.tensor_copy(om[:], po[:])
                    nc.sync.dma_start(
                        out[t0 + tb * P : t0 + (tb + 1) * P, m * 512 : (m + 1) * 512],
                        om[:],
                    )
    nc.compile()
    return nc


def _get_program(t_pad: int, nt: int):
    key = (t_pad, nt)
    if key not in _PROGRAM_CACHE:
        _PROGRAM_CACHE[key] = _build_program(t_pad, nt)
    return _PROGRAM_CACHE[key]


def _pack_w1(w: np.ndarray) -> np.ndarray:
    # [HIDDEN, GU] f32 -> [KH, P, GU] bf16 (pure reshape + cast)
    return w.reshape(KH, P, GU).astype(BF16)


def _pack_w2(w: np.ndarray) -> np.ndarray:
    # [INTER, HIDDEN] f32 -> [KI, P, HIDDEN] bf16 (pure reshape + cast)
    return w.reshape(KI, P, HIDDEN).astype(BF16)


def _assign_experts(counts):
    """Expert e -> core (e % N_CORES); slots stack when E > N_CORES."""
    n_exp = len(counts)
    n_slots = max(1, math.ceil(n_exp / N_CORES))
    cores = []
    for r in range(N_CORES):
        slots = []
        for s in range(n_slots):
            e = s * N_CORES + r
            slots.append(e if e < n_exp else None)
        cores.append(slots)
    return cores, n_slots


def _run(
    hidden_states: np.ndarray,
    merged_gate_up_proj: np.ndarray,
    merged_down_proj: np.ndarray,
    num_tokens_per_expert: np.ndarray,
    trace: bool = False,
):
    counts = [int(c) for c in np.asarray(num_tokens_per_expert)]
    n_exp = len(counts)
    offs = np.concatenate([[0], np.cumsum(counts)]).astype(int)
    total = int(offs[-1])

    core_experts, n_slots = _assign_experts(counts)
    assert n_slots == 1, "more experts than cores not supported"
    per_core_rows = [sum(counts[e] for e in slots if e is not None)
                     for slots in core_experts]
    t_pad = max(NT, ((max(per_core_rows) + NT - 1) // NT) * NT)

    nc = _get_program(t_pad, NT)

    from concurrent.futures import ThreadPoolExecutor

    pool = ThreadPoolExecutor(8)

    # [TOTAL, HIDDEN] f32 -> bf16 -> transposed [HIDDEN, TOTAL] -> [KH, P, TOTAL]
    x_bf16 = hidden_states[:total].astype(BF16)
    xT_full = np.empty((HIDDEN, total), dtype=BF16)

    def _tr(k):
        xT_full[k * P : (k + 1) * P] = x_bf16[:, k * P : (k + 1) * P].T

    list(pool.map(_tr, range(KH)))
    xT_full = xT_full.reshape(KH, P, total)

    w1_packed = list(pool.map(
        lambda e: _pack_w1(merged_gate_up_proj[e]), range(n_exp)
    ))
    w2_packed = list(pool.map(
        lambda e: _pack_w2(merged_down_proj[e]), range(n_exp)
    ))

    def _core_x(r):
        e = core_experts[r][0]
        xT_core = np.zeros((KH, P, t_pad), dtype=BF16)
        if e is not None and counts[e]:
            xT_core[:, :, : counts[e]] = xT_full[:, :, offs[e] : offs[e] + counts[e]]
        return xT_core

    core_x = list(pool.map(_core_x, range(N_CORES)))
    pool.shutdown(wait=True)

    in_maps = []
    for r in range(N_CORES):
        e = core_experts[r][0]
        ew = e if (e is not None and e < n_exp) else 0
        in_maps.append(
            {"xT": core_x[r], "w1": w1_packed[ew], "w2": w2_packed[ew]}
        )

    res = _execute(nc, in_maps, trace)

    out = np.empty((total, HIDDEN), dtype=np.float32)
    for r in range(N_CORES):
        e = core_experts[r][0]
        if e is None or counts[e] == 0:
            continue
        cnt = counts[e]
        o_core = res.results[r]["out"]
        out[offs[e] : offs[e] + cnt] = o_core[:cnt].astype(np.float32)
    return out, res


def _execute(nc, in_maps, trace):
    from concourse.bass_utils import run_bass_kernel_spmd

    if not trace:
        try:
            return _execute_pjrt_dev_zeros(nc, in_maps)
        except Exception:
            pass
    # "out" and "xT" have identical byte sizes; the kernel's semaphore
    # chain guarantees every x row is consumed before the corresponding
    # out row is stored, so donating xT's device buffer to out is safe.
    # (On the axon path aliases are ignored; on the native path this
    # skips staging a zero buffer for the output.)
    return run_bass_kernel_spmd(
        nc, in_maps, list(range(N_CORES)), aliases={"out": "xT"}, trace=trace
    )


_EXEC_CACHE: dict = {}


def _build_pjrt_executor(nc):
    from concourse.bass_utils import axon_active
    import concourse.mybir as mybir
    from concourse import bass2jax
    import jax
    import jax.numpy as jnp
    from jax.sharding import Mesh, PartitionSpec, NamedSharding
    from jax.experimental.shard_map import shard_map

    if not axon_active():
        raise RuntimeError("pjrt path requires axon")
    if nc.dbg_addr is not None:
        raise RuntimeError("debug program")

    bass2jax.install_neuronx_cc_hook()

    partition_name = nc.partition_id_tensor.name if nc.partition_id_tensor else None
    in_names, out_names, out_avals = [], [], []
    for alloc in nc.m.functions[0].allocations:
        if not isinstance(alloc, mybir.MemoryLocationSet):
            continue
        name = alloc.memorylocations[0].name
        if alloc.kind == "ExternalInput":
            if name != partition_name:
                in_names.append(name)
        elif alloc.kind == "ExternalOutput":
            out_names.append(name)
            out_avals.append(
                jax.core.ShapedArray(
                    tuple(alloc.tensor_shape), mybir.dt.np(alloc.dtype)
                )
            )
    n_params = len(in_names)
    n_outs = len(out_avals)
    all_names = in_names + out_names
    if partition_name is not None:
        all_names = all_names + [partition_name]
    donate = tuple(range(n_params, n_params + n_outs))

    def _body(*args):
        operands = list(args)
        if partition_name is not None:
            operands.append(bass2jax.partition_id_tensor())
        outs = bass2jax._bass_exec_p.bind(
            *operands,
            out_avals=tuple(out_avals),
            in_names=tuple(all_names),
            out_names=tuple(out_names),
            lowering_input_output_aliases=(),
            sim_require_finite=True,
            sim_require_nnan=True,
            nc=nc,
        )
        return tuple(outs)

    devices = jax.devices()[:N_CORES]
    assert len(devices) == N_CORES
    mesh = Mesh(np.asarray(devices), ("core",))
    in_specs = (PartitionSpec("core"),) * (n_params + n_outs)
    out_specs = (PartitionSpec("core"),) * n_outs
    sharded = jax.jit(
        shard_map(
            _body, mesh=mesh, in_specs=in_specs, out_specs=out_specs,
            check_rep=False,
        ),
        donate_argnums=donate,
        keep_unused=True,
    )
    zsharding = NamedSharding(mesh, PartitionSpec("core"))
    zero_fns = [
        jax.jit(
            lambda s=av.shape, d=av.dtype: jnp.zeros(
                (N_CORES * s[0], *s[1:]), d
            ),
            out_shardings=zsharding,
        )
        for av in out_avals
    ]
    return {
        "sharded": sharded,
        "zero_fns": zero_fns,
        "in_names": in_names,
        "out_names": out_names,
        "out_avals": out_avals,
    }


def _execute_pjrt_dev_zeros(nc, in_maps):
    """run_bass_via_pjrt equivalent, but the donated zero output buffers
    are created on-device (jnp.zeros under jit) instead of being staged
    from host numpy -- saves shipping one full output-sized array of
    zeros per core over the host->device link."""
    from concourse.bass_utils import BassKernelResults

    key = id(nc)
    if key not in _EXEC_CACHE:
        _EXEC_CACHE[key] = _build_pjrt_executor(nc)
    ex = _EXEC_CACHE[key]

    concat_in = [
        np.concatenate([np.asarray(m[name]) for m in in_maps], axis=0)
        for name in ex["in_names"]
    ]
    dev_zeros = [fn() for fn in ex["zero_fns"]]
    out_arrs = ex["sharded"](*concat_in, *dev_zeros)
    out_avals = ex["out_avals"]
    results = [
        {
            name: np.asarray(out_arrs[i]).reshape(
                N_CORES, *out_avals[i].shape
            )[c]
            for i, name in enumerate(ex["out_names"])
        }
        for c in range(N_CORES)
    ]
    return BassKernelResults(
        results=results,
        instructions_and_trace=None,
        profile_json=None,
        exec_time_ns=None,
    )


def kernel(**inputs) -> np.ndarray:
    return _run(**inputs, trace=False)[0]


def run_traced(**inputs):
    return _run(**inputs, trace=True)


# revision 12
# speedup vs baseline: 1.0150x; 1.0051x over previous
"""Grouped MoE MLP (SwiGLU) kernel for Trainium2, 8 NeuronCores.

Strategy (pure expert-parallel):
  Tokens arrive pre-sorted by expert with per-expert counts.  Expert e's
  weights and token block go to core e (one expert per core, no weight
  duplication -- weight bytes shipped to the device are the dominant cost
  for this problem).  Token blocks are zero-padded to a common T_pad so
  all cores run one SPMD program.

  Device program per core (dense SwiGLU MLP over T_pad tokens):
    GEMM1: h1^T[f, t] = sum_h W1[h, f] * x[t, h]     (h on partitions)
    SwiGLU on feature-partitioned tiles
    GEMM2: out[t, o]  = sum_f h[t, f] * W2[f, o]     (f on partitions,
           tokens become the PSUM partition dim so the output lands in
           natural [T, HIDDEN] layout -- no transposes on the way out)

  All device I/O is bf16 (inputs cast on host, output cast back to f32
  on host) to halve the bytes staged over the host<->device link.
"""

import math
from contextlib import ExitStack

import ml_dtypes
import numpy as np

P = 128
HIDDEN = 2048
INTER = 1408
GU = 2 * INTER            # 2816 = gate+up columns
KH = HIDDEN // P          # 16 k-tiles for GEMM1
KI = INTER // P           # 11 k-tiles for GEMM2 / gate-up pair blocks
NO = HIDDEN // 512        # 4 output column blocks of 512
N_CORES = 8
NT = 512                  # tokens per chunk (matmul moving free dim)

BF16 = ml_dtypes.bfloat16

_PROGRAM_CACHE: dict = {}


def _build_program(t_pad: int, nt: int):
    import concourse.mybir as mybir
    import concourse.tile as tile
    from concourse import bacc

    bf16 = mybir.dt.bfloat16
    f32 = mybir.dt.float32

    n_chunks = t_pad // nt
    nb = nt // P

    nc = bacc.Bacc(None, target_bir_lowering=False, debug=False)
    # chunk-major layouts: every DMA below moves one fully contiguous
    # DRAM block (x loads 128 KiB, w1 704 KiB, w2 512 KiB, out stores
    # 128 KiB) -- friendly to any memory system the tensors live in.
    xT = nc.dram_tensor("xT", [n_chunks, KH, P, nt], bf16, kind="ExternalInput")
    w1 = nc.dram_tensor("w1", [KH, P, GU], bf16, kind="ExternalInput")
    w2 = nc.dram_tensor("w2", [KI, P, HIDDEN], bf16, kind="ExternalInput")
    out = nc.dram_tensor(
        "out", [n_chunks, nb, NO, P, 512], bf16, kind="ExternalOutput"
    )

    with tile.TileContext(nc) as tc, ExitStack() as ctx:
        w1_pool = ctx.enter_context(tc.tile_pool(name="w1p", bufs=1))
        w2_pool = ctx.enter_context(tc.tile_pool(name="w2p", bufs=1))
        x_pool = ctx.enter_context(tc.tile_pool(name="xp", bufs=2))
        h_pool = ctx.enter_context(tc.tile_pool(name="hp", bufs=2))
        g_pool = ctx.enter_context(tc.tile_pool(name="gp", bufs=2))
        o_pool = ctx.enter_context(tc.tile_pool(name="op", bufs=3))
        ps1 = ctx.enter_context(tc.tile_pool(name="ps1", bufs=2, space="PSUM"))
        ps2 = ctx.enter_context(tc.tile_pool(name="ps2", bufs=2, space="PSUM"))

        # first x chunk goes ahead of the weight DMAs so the first GEMM1
        # matmul only waits for w1, not the whole weight set (HWDGE
        # queues are FIFO)
        xt0 = x_pool.tile([P, KH, nt], bf16, tag="xt")
        for k in range(KH):
            nc.sync.dma_start(xt0[:, k, :], xT[0, k])
        w1t = w1_pool.tile([P, KH, GU], bf16)
        for k in range(KH):
            nc.sync.dma_start(w1t[:, k, :], w1[k])
        w2t = w2_pool.tile([P, KI, HIDDEN], bf16)
        for k in range(KI):
            nc.sync.dma_start(w2t[:, k, :], w2[k])

        c_n = nt
        for ci in range(n_chunks):
            if ci == 0:
                xt = xt0
            else:
                xt = x_pool.tile([P, KH, c_n], bf16, tag="xt")
                for k in range(KH):
                    nc.sync.dma_start(xt[:, k, :], xT[ci, k])
            ht = h_pool.tile([P, KI, c_n], bf16, tag="ht")
            for mp in range(KI):
                pg = ps1.tile([P, c_n], f32, tag="pg")
                pu = ps1.tile([P, c_n], f32, tag="pu")
                for k in range(KH):
                    nc.tensor.matmul(
                        pg[:],
                        w1t[:, k, mp * P : (mp + 1) * P],
                        xt[:, k, :],
                        start=(k == 0),
                        stop=(k == KH - 1),
                    )
                for k in range(KH):
                    nc.tensor.matmul(
                        pu[:],
                        w1t[:, k, (KI + mp) * P : (KI + mp + 1) * P],
                        xt[:, k, :],
                        start=(k == 0),
                        stop=(k == KH - 1),
                    )
                gt = g_pool.tile([P, c_n], bf16, tag="gt")
                nc.scalar.activation(
                    gt[:], pg[:], mybir.ActivationFunctionType.Silu
                )
                nc.vector.tensor_mul(ht[:, mp, :], gt[:], pu[:])
            # GEMM2 with tokens on the PSUM partition dim: for each
            # 128-token block, out[tb, o_blk] = ht[:, :, tb].T @ w2
            for tb in range(c_n // P):
                for m in range(NO):
                    po = ps2.tile([P, 512], f32, tag="po")
                    for k in range(KI):
                        nc.tensor.matmul(
                            po[:],
                            ht[:, k, tb * P : (tb + 1) * P],
                            w2t[:, k, m * 512 : (m + 1) * 512],
                            start=(k == 0),
                            stop=(k == KI - 1),
                        )
                    om = o_pool.tile([P, 512], bf16, tag="om")
                    nc.vector.tensor_copy(om[:], po[:])
                    nc.sync.dma_start(out[ci, tb, m], om[:])
    nc.compile()
    return nc


def _get_program(t_pad: int, nt: int):
    key = (t_pad, nt)
    if key not in _PROGRAM_CACHE:
        _PROGRAM_CACHE[key] = _build_program(t_pad, nt)
    return _PROGRAM_CACHE[key]


def _pack_w1(w: np.ndarray) -> np.ndarray:
    # [HIDDEN, GU] f32 -> [KH, P, GU] bf16 (pure reshape + cast)
    return w.reshape(KH, P, GU).astype(BF16)


def _pack_w2(w: np.ndarray) -> np.ndarray:
    # [INTER, HIDDEN] f32 -> [KI, P, HIDDEN] bf16 (pure reshape + cast)
    return w.reshape(KI, P, HIDDEN).astype(BF16)


def _assign_experts(counts):
    """Expert e -> core (e % N_CORES); slots stack when E > N_CORES."""
    n_exp = len(counts)
    n_slots = max(1, math.ceil(n_exp / N_CORES))
    cores = []
    for r in range(N_CORES):
        slots = []
        for s in range(n_slots):
            e = s * N_CORES + r
            slots.append(e if e < n_exp else None)
        cores.append(slots)
    return cores, n_slots


def _run(
    hidden_states: np.ndarray,
    merged_gate_up_proj: np.ndarray,
    merged_down_proj: np.ndarray,
    num_tokens_per_expert: np.ndarray,
    trace: bool = False,
):
    counts = [int(c) for c in np.asarray(num_tokens_per_expert)]
    n_exp = len(counts)
    offs = np.concatenate([[0], np.cumsum(counts)]).astype(int)
    total = int(offs[-1])

    core_experts, n_slots = _assign_experts(counts)
    assert n_slots == 1, "more experts than cores not supported"
    per_core_rows = [sum(counts[e] for e in slots if e is not None)
                     for slots in core_experts]
    t_pad = max(NT, ((max(per_core_rows) + NT - 1) // NT) * NT)

    nc = _get_program(t_pad, NT)

    from concurrent.futures import ThreadPoolExecutor

    pool = ThreadPoolExecutor(8)

    # [TOTAL, HIDDEN] f32 -> bf16 -> transposed [HIDDEN, TOTAL] -> [KH, P, TOTAL]
    x_bf16 = hidden_states[:total].astype(BF16)
    xT_full = np.empty((HIDDEN, total), dtype=BF16)

    def _tr(k):
        xT_full[k * P : (k + 1) * P] = x_bf16[:, k * P : (k + 1) * P].T

    list(pool.map(_tr, range(KH)))
    xT_full = xT_full.reshape(KH, P, total)

    w1_packed = list(pool.map(
        lambda e: _pack_w1(merged_gate_up_proj[e]), range(n_exp)
    ))
    w2_packed = list(pool.map(
        lambda e: _pack_w2(merged_down_proj[e]), range(n_exp)
    ))

    n_chunks = t_pad // NT

    def _core_x(r):
        e = core_experts[r][0]
        xT_core = np.zeros((n_chunks, KH, P, NT), dtype=BF16)
        if e is not None and counts[e]:
            cnt = counts[e]
            for ci in range(n_chunks):
                t0 = ci * NT
                n = min(NT, cnt - t0)
                if n <= 0:
                    break
                xT_core[ci, :, :, :n] = xT_full[
                    :, :, offs[e] + t0 : offs[e] + t0 + n
                ]
        return xT_core

    core_x = list(pool.map(_core_x, range(N_CORES)))
    pool.shutdown(wait=True)

    in_maps = []
    for r in range(N_CORES):
        e = core_experts[r][0]
        ew = e if (e is not None and e < n_exp) else 0
        in_maps.append(
            {"xT": core_x[r], "w1": w1_packed[ew], "w2": w2_packed[ew]}
        )

    res = _execute(nc, in_maps, trace)

    out = np.empty((total, HIDDEN), dtype=np.float32)

    def _unshard(r):
        e = core_experts[r][0]
        if e is None or counts[e] == 0:
            return
        cnt = counts[e]
        # [n_chunks, nb, NO, P, 512] -> [t, o] with t = (ci, tb, p),
        # o = (m, j)
        o_core = np.ascontiguousarray(
            res.results[r]["out"].transpose(0, 1, 3, 2, 4)
        ).reshape(t_pad, HIDDEN)
        out[offs[e] : offs[e] + cnt] = o_core[:cnt].astype(np.float32)

    upool = ThreadPoolExecutor(8)
    list(upool.map(_unshard, range(N_CORES)))
    upool.shutdown(wait=True)
    return out, res


def _execute(nc, in_maps, trace):
    from concourse.bass_utils import run_bass_kernel_spmd

    if not trace:
        try:
            return _execute_pjrt_dev_zeros(nc, in_maps)
        except Exception:
            pass
    # "out" and "xT" have identical byte sizes; the kernel's semaphore
    # chain guarantees every x row is consumed before the corresponding
    # out row is stored, so donating xT's device buffer to out is safe.
    # (On the axon path aliases are ignored; on the native path this
    # skips staging a zero buffer for the output.)
    return run_bass_kernel_spmd(
        nc, in_maps, list(range(N_CORES)), aliases={"out": "xT"}, trace=trace
    )


_EXEC_CACHE: dict = {}


def _build_pjrt_executor(nc):
    from concourse.bass_utils import axon_active
    import concourse.mybir as mybir
    from concourse import bass2jax
    import jax
    import jax.numpy as jnp
    from jax.sharding import Mesh, PartitionSpec, NamedSharding
    from jax.experimental.shard_map import shard_map

    if not axon_active():
        raise RuntimeError("pjrt path requires axon")
    if nc.dbg_addr is not None:
        raise RuntimeError("debug program")

    bass2jax.install_neuronx_cc_hook()

    partition_name = nc.partition_id_tensor.name if nc.partition_id_tensor else None
    in_names, out_names, out_avals = [], [], []
    for alloc in nc.m.functions[0].allocations:
        if not isinstance(alloc, mybir.MemoryLocationSet):
            continue
        name = alloc.memorylocations[0].name
        if alloc.kind == "ExternalInput":
            if name != partition_name:
                in_names.append(name)
        elif alloc.kind == "ExternalOutput":
            out_names.append(name)
            out_avals.append(
                jax.core.ShapedArray(
                    tuple(alloc.tensor_shape), mybir.dt.np(alloc.dtype)
                )
            )
    n_params = len(in_names)
    n_outs = len(out_avals)
    all_names = in_names + out_names
    if partition_name is not None:
        all_names = all_names + [partition_name]
    donate = tuple(range(n_params, n_params + n_outs))

    def _body(*args):
        operands = list(args)
        if partition_name is not None:
            operands.append(bass2jax.partition_id_tensor())
        outs = bass2jax._bass_exec_p.bind(
            *operands,
            out_avals=tuple(out_avals),
            in_names=tuple(all_names),
            out_names=tuple(out_names),
            lowering_input_output_aliases=(),
            sim_require_finite=True,
            sim_require_nnan=True,
            nc=nc,
        )
        return tuple(outs)

    devices = jax.devices()[:N_CORES]
    assert len(devices) == N_CORES
    mesh = Mesh(np.asarray(devices), ("core",))
    in_specs = (PartitionSpec("core"),) * (n_params + n_outs)
    out_specs = (PartitionSpec("core"),) * n_outs
    sharded = jax.jit(
        shard_map(
            _body, mesh=mesh, in_specs=in_specs, out_specs=out_specs,
            check_rep=False,
        ),
        donate_argnums=donate,
        keep_unused=True,
    )
    zsharding = NamedSharding(mesh, PartitionSpec("core"))
    zero_fns = [
        jax.jit(
            lambda s=av.shape, d=av.dtype: jnp.zeros(
                (N_CORES * s[0], *s[1:]), d
            ),
            out_shardings=zsharding,
        )
        for av in out_avals
    ]
    return {
        "sharded": sharded,
        "zero_fns": zero_fns,
        "in_names": in_names,
        "out_names": out_names,
        "out_avals": out_avals,
    }


def _execute_pjrt_dev_zeros(nc, in_maps):
    """run_bass_via_pjrt equivalent, but the donated zero output buffers
    are created on-device (jnp.zeros under jit) instead of being staged
    from host numpy -- saves shipping one full output-sized array of
    zeros per core over the host->device link."""
    from concourse.bass_utils import BassKernelResults

    key = id(nc)
    if key not in _EXEC_CACHE:
        _EXEC_CACHE[key] = _build_pjrt_executor(nc)
    ex = _EXEC_CACHE[key]

    concat_in = [
        np.concatenate([np.asarray(m[name]) for m in in_maps], axis=0)
        for name in ex["in_names"]
    ]
    dev_zeros = [fn() for fn in ex["zero_fns"]]
    out_arrs = ex["sharded"](*concat_in, *dev_zeros)
    out_avals = ex["out_avals"]
    results = [
        {
            name: np.asarray(out_arrs[i]).reshape(
                N_CORES, *out_avals[i].shape
            )[c]
            for i, name in enumerate(ex["out_names"])
        }
        for c in range(N_CORES)
    ]
    return BassKernelResults(
        results=results,
        instructions_and_trace=None,
        profile_json=None,
        exec_time_ns=None,
    )


def kernel(**inputs) -> np.ndarray:
    return _run(**inputs, trace=False)[0]


def run_traced(**inputs):
    return _run(**inputs, trace=True)


# revision 16
# speedup vs baseline: 1.0308x; 1.0156x over previous
"""Grouped MoE MLP (SwiGLU) kernel for Trainium2, 8 NeuronCores.

Strategy (pure expert-parallel):
  Tokens arrive pre-sorted by expert with per-expert counts.  Expert e's
  weights and token block go to core e (one expert per core, no weight
  duplication -- weight bytes shipped to the device are the dominant cost
  for this problem).  Token blocks are zero-padded to a common T_pad so
  all cores run one SPMD program.

  Device program per core (dense SwiGLU MLP over T_pad tokens):
    GEMM1: h1^T[f, t] = sum_h W1[h, f] * x[t, h]     (h on partitions)
    SwiGLU on feature-partitioned tiles
    GEMM2: out[t, o]  = sum_f h[t, f] * W2[f, o]     (f on partitions,
           tokens become the PSUM partition dim so the output lands in
           natural [T, HIDDEN] layout -- no transposes on the way out)

  All device I/O is bf16 (inputs cast on host, output cast back to f32
  on host) to halve the bytes staged over the host<->device link.
"""

import math
from contextlib import ExitStack

import ml_dtypes
import numpy as np

P = 128
HIDDEN = 2048
INTER = 1408
GU = 2 * INTER            # 2816 = gate+up columns
KH = HIDDEN // P          # 16 k-tiles for GEMM1
KI = INTER // P           # 11 k-tiles for GEMM2 / gate-up pair blocks
NO = HIDDEN // 512        # 4 output column blocks of 512
N_CORES = 8
NT = 512                  # tokens per chunk (matmul moving free dim)

BF16 = ml_dtypes.bfloat16

_PROGRAM_CACHE: dict = {}


def _build_program(t_pad: int, nt: int):
    import concourse.mybir as mybir
    import concourse.tile as tile
    from concourse import bacc

    bf16 = mybir.dt.bfloat16
    f32 = mybir.dt.float32

    n_chunks = t_pad // nt
    nb = nt // P

    nc = bacc.Bacc(None, target_bir_lowering=False, debug=False)
    # chunk-major layouts: every DMA below moves one fully contiguous
    # DRAM block (x loads 128 KiB, w1 704 KiB, w2 512 KiB, out stores
    # 128 KiB) -- friendly to any memory system the tensors live in.
    xT = nc.dram_tensor("xT", [n_chunks, KH, P, nt], bf16, kind="ExternalInput")
    # w1 is column-group-major: group g < KI is gate cols [128g, 128g+128),
    # group KI+g is the matching up block -- so GEMM1 group mp only waits
    # for its own two 720 KiB blocks, not the whole 11.5 MiB tensor.
    w1 = nc.dram_tensor("w1", [2 * KI, P, KH, P], bf16, kind="ExternalInput")
    w2 = nc.dram_tensor("w2", [KI, P, HIDDEN], bf16, kind="ExternalInput")
    out = nc.dram_tensor(
        "out", [n_chunks, nb, NO, P, 512], bf16, kind="ExternalOutput"
    )

    with tile.TileContext(nc) as tc, ExitStack() as ctx:
        w1_pool = ctx.enter_context(tc.tile_pool(name="w1p", bufs=1))
        w2_pool = ctx.enter_context(tc.tile_pool(name="w2p", bufs=1))
        x_pool = ctx.enter_context(tc.tile_pool(name="xp", bufs=2))
        h_pool = ctx.enter_context(tc.tile_pool(name="hp", bufs=2))
        g_pool = ctx.enter_context(tc.tile_pool(name="gp", bufs=2))
        o_pool = ctx.enter_context(tc.tile_pool(name="op", bufs=3))
        ps1 = ctx.enter_context(tc.tile_pool(name="ps1", bufs=2, space="PSUM"))
        ps2 = ctx.enter_context(tc.tile_pool(name="ps2", bufs=2, space="PSUM"))

        # first x chunk goes ahead of the weight DMAs so the first GEMM1
        # matmul only waits for w1, not the whole weight set (HWDGE
        # queues are FIFO)
        xt0 = x_pool.tile([P, KH, nt], bf16, tag="xt")
        for k in range(KH):
            nc.sync.dma_start(xt0[:, k, :], xT[0, k])
        w1t = w1_pool.tile([P, 2 * KI, KH, P], bf16)
        # interleave gate/up group DMAs in mp order so group mp's two
        # blocks land just before its matmuls need them
        for mp in range(KI):
            nc.sync.dma_start(w1t[:, mp], w1[mp])
            nc.sync.dma_start(w1t[:, KI + mp], w1[KI + mp])
        w2t = w2_pool.tile([P, KI, HIDDEN], bf16)
        for k in range(KI):
            nc.sync.dma_start(w2t[:, k, :], w2[k])

        c_n = nt
        for ci in range(n_chunks):
            if ci == 0:
                xt = xt0
            else:
                xt = x_pool.tile([P, KH, c_n], bf16, tag="xt")
                for k in range(KH):
                    nc.sync.dma_start(xt[:, k, :], xT[ci, k])
            ht = h_pool.tile([P, KI, c_n], bf16, tag="ht")
            for mp in range(KI):
                pg = ps1.tile([P, c_n], f32, tag="pg")
                pu = ps1.tile([P, c_n], f32, tag="pu")
                for k in range(KH):
                    nc.tensor.matmul(
                        pg[:],
                        w1t[:, mp, k, :],
                        xt[:, k, :],
                        start=(k == 0),
                        stop=(k == KH - 1),
                    )
                for k in range(KH):
                    nc.tensor.matmul(
                        pu[:],
                        w1t[:, KI + mp, k, :],
                        xt[:, k, :],
                        start=(k == 0),
                        stop=(k == KH - 1),
                    )
                gt = g_pool.tile([P, c_n], bf16, tag="gt")
                nc.scalar.activation(
                    gt[:], pg[:], mybir.ActivationFunctionType.Silu
                )
                nc.vector.tensor_mul(ht[:, mp, :], gt[:], pu[:])
            # GEMM2 with tokens on the PSUM partition dim: for each
            # 128-token block, out[tb, o_blk] = ht[:, :, tb].T @ w2
            for tb in range(c_n // P):
                for m in range(NO):
                    po = ps2.tile([P, 512], f32, tag="po")
                    for k in range(KI):
                        nc.tensor.matmul(
                            po[:],
                            ht[:, k, tb * P : (tb + 1) * P],
                            w2t[:, k, m * 512 : (m + 1) * 512],
                            start=(k == 0),
                            stop=(k == KI - 1),
                        )
                    om = o_pool.tile([P, 512], bf16, tag="om")
                    nc.vector.tensor_copy(om[:], po[:])
                    nc.sync.dma_start(out[ci, tb, m], om[:])
    nc.compile()
    return nc


def _get_program(t_pad: int, nt: int):
    key = (t_pad, nt)
    if key not in _PROGRAM_CACHE:
        _PROGRAM_CACHE[key] = _build_program(t_pad, nt)
    return _PROGRAM_CACHE[key]


def _pack_w1(w: np.ndarray) -> np.ndarray:
    # [HIDDEN, GU] f32 -> column-group-major [2*KI, P, KH, 128] bf16
    # (row h = 128k + p, col c = 128g + j)
    return w.reshape(KH, P, 2 * KI, P).transpose(2, 1, 0, 3).astype(BF16)


def _pack_w2(w: np.ndarray) -> np.ndarray:
    # [INTER, HIDDEN] f32 -> [KI, P, HIDDEN] bf16 (pure reshape + cast)
    return w.reshape(KI, P, HIDDEN).astype(BF16)


def _assign_experts(counts):
    """Expert e -> core (e % N_CORES); slots stack when E > N_CORES."""
    n_exp = len(counts)
    n_slots = max(1, math.ceil(n_exp / N_CORES))
    cores = []
    for r in range(N_CORES):
        slots = []
        for s in range(n_slots):
            e = s * N_CORES + r
            slots.append(e if e < n_exp else None)
        cores.append(slots)
    return cores, n_slots


def _run(
    hidden_states: np.ndarray,
    merged_gate_up_proj: np.ndarray,
    merged_down_proj: np.ndarray,
    num_tokens_per_expert: np.ndarray,
    trace: bool = False,
):
    counts = [int(c) for c in np.asarray(num_tokens_per_expert)]
    n_exp = len(counts)
    offs = np.concatenate([[0], np.cumsum(counts)]).astype(int)
    total = int(offs[-1])

    core_experts, n_slots = _assign_experts(counts)
    assert n_slots == 1, "more experts than cores not supported"
    per_core_rows = [sum(counts[e] for e in slots if e is not None)
                     for slots in core_experts]
    t_pad = max(NT, ((max(per_core_rows) + NT - 1) // NT) * NT)

    nc = _get_program(t_pad, NT)

    from concurrent.futures import ThreadPoolExecutor

    pool = ThreadPoolExecutor(8)

    # [TOTAL, HIDDEN] f32 -> bf16 -> transposed [HIDDEN, TOTAL] -> [KH, P, TOTAL]
    x_bf16 = hidden_states[:total].astype(BF16)
    xT_full = np.empty((HIDDEN, total), dtype=BF16)

    def _tr(k):
        xT_full[k * P : (k + 1) * P] = x_bf16[:, k * P : (k + 1) * P].T

    list(pool.map(_tr, range(KH)))
    xT_full = xT_full.reshape(KH, P, total)

    w1_packed = list(pool.map(
        lambda e: _pack_w1(merged_gate_up_proj[e]), range(n_exp)
    ))
    w2_packed = list(pool.map(
        lambda e: _pack_w2(merged_down_proj[e]), range(n_exp)
    ))

    n_chunks = t_pad // NT

    def _core_x(r):
        e = core_experts[r][0]
        xT_core = np.zeros((n_chunks, KH, P, NT), dtype=BF16)
        if e is not None and counts[e]:
            cnt = counts[e]
            for ci in range(n_chunks):
                t0 = ci * NT
                n = min(NT, cnt - t0)
                if n <= 0:
                    break
                xT_core[ci, :, :, :n] = xT_full[
                    :, :, offs[e] + t0 : offs[e] + t0 + n
                ]
        return xT_core

    core_x = list(pool.map(_core_x, range(N_CORES)))
    pool.shutdown(wait=True)

    in_maps = []
    for r in range(N_CORES):
        e = core_experts[r][0]
        ew = e if (e is not None and e < n_exp) else 0
        in_maps.append(
            {"xT": core_x[r], "w1": w1_packed[ew], "w2": w2_packed[ew]}
        )

    res = _execute(nc, in_maps, trace)

    out = np.empty((total, HIDDEN), dtype=np.float32)

    def _unshard(r):
        e = core_experts[r][0]
        if e is None or counts[e] == 0:
            return
        cnt = counts[e]
        # [n_chunks, nb, NO, P, 512] -> [t, o] with t = (ci, tb, p),
        # o = (m, j)
        o_core = np.ascontiguousarray(
            res.results[r]["out"].transpose(0, 1, 3, 2, 4)
        ).reshape(t_pad, HIDDEN)
        out[offs[e] : offs[e] + cnt] = o_core[:cnt].astype(np.float32)

    upool = ThreadPoolExecutor(8)
    list(upool.map(_unshard, range(N_CORES)))
    upool.shutdown(wait=True)
    return out, res


def _execute(nc, in_maps, trace):
    from concourse.bass_utils import run_bass_kernel_spmd

    if not trace:
        try:
            return _execute_pjrt_dev_zeros(nc, in_maps)
        except Exception:
            pass
    # "out" and "xT" have identical byte sizes; the kernel's semaphore
    # chain guarantees every x row is consumed before the corresponding
    # out row is stored, so donating xT's device buffer to out is safe.
    # (On the axon path aliases are ignored; on the native path this
    # skips staging a zero buffer for the output.)
    return run_bass_kernel_spmd(
        nc, in_maps, list(range(N_CORES)), aliases={"out": "xT"}, trace=trace
    )


_EXEC_CACHE: dict = {}


def _build_pjrt_executor(nc):
    from concourse.bass_utils import axon_active
    import concourse.mybir as mybir
    from concourse import bass2jax
    import jax
    import jax.numpy as jnp
    from jax.sharding import Mesh, PartitionSpec, NamedSharding
    from jax.experimental.shard_map import shard_map

    if not axon_active():
        raise RuntimeError("pjrt path requires axon")
    if nc.dbg_addr is not None:
        raise RuntimeError("debug program")

    bass2jax.install_neuronx_cc_hook()

    partition_name = nc.partition_id_tensor.name if nc.partition_id_tensor else None
    in_names, out_names, out_avals = [], [], []
    for alloc in nc.m.functions[0].allocations:
        if not isinstance(alloc, mybir.MemoryLocationSet):
            continue
        name = alloc.memorylocations[0].name
        if alloc.kind == "ExternalInput":
            if name != partition_name:
                in_names.append(name)
        elif alloc.kind == "ExternalOutput":
            out_names.append(name)
            out_avals.append(
                jax.core.ShapedArray(
                    tuple(alloc.tensor_shape), mybir.dt.np(alloc.dtype)
                )
            )
    n_params = len(in_names)
    n_outs = len(out_avals)
    all_names = in_names + out_names
    if partition_name is not None:
        all_names = all_names + [partition_name]
    donate = tuple(range(n_params, n_params + n_outs))

    def _body(*args):
        operands = list(args)
        if partition_name is not None:
            operands.append(bass2jax.partition_id_tensor())
        outs = bass2jax._bass_exec_p.bind(
            *operands,
            out_avals=tuple(out_avals),
            in_names=tuple(all_names),
            out_names=tuple(out_names),
            lowering_input_output_aliases=(),
            sim_require_finite=True,
            sim_require_nnan=True,
            nc=nc,
        )
        return tuple(outs)

    devices = jax.devices()[:N_CORES]
    assert len(devices) == N_CORES
    mesh = Mesh(np.asarray(devices), ("core",))
    in_specs = (PartitionSpec("core"),) * (n_params + n_outs)
    out_specs = (PartitionSpec("core"),) * n_outs
    sharded = jax.jit(
        shard_map(
            _body, mesh=mesh, in_specs=in_specs, out_specs=out_specs,
            check_rep=False,
        ),
        donate_argnums=donate,
        keep_unused=True,
    )
    zsharding = NamedSharding(mesh, PartitionSpec("core"))
    zero_fns = [
        jax.jit(
            lambda s=av.shape, d=av.dtype: jnp.zeros(
                (N_CORES * s[0], *s[1:]), d
            ),
            out_shardings=zsharding,
        )
        for av in out_avals
    ]
    return {
        "sharded": sharded,
        "zero_fns": zero_fns,
        "in_names": in_names,
        "out_names": out_names,
        "out_avals": out_avals,
    }


def _execute_pjrt_dev_zeros(nc, in_maps):
    """run_bass_via_pjrt equivalent, but the donated zero output buffers
    are created on-device (jnp.zeros under jit) instead of being staged
    from host numpy -- saves shipping one full output-sized array of
    zeros per core over the host->device link."""
    from concourse.bass_utils import BassKernelResults

    key = id(nc)
    if key not in _EXEC_CACHE:
        _EXEC_CACHE[key] = _build_pjrt_executor(nc)
    ex = _EXEC_CACHE[key]

    concat_in = [
        np.concatenate([np.asarray(m[name]) for m in in_maps], axis=0)
        for name in ex["in_names"]
    ]
    dev_zeros = [fn() for fn in ex["zero_fns"]]
    out_arrs = ex["sharded"](*concat_in, *dev_zeros)
    out_avals = ex["out_avals"]
    results = [
        {
            name: np.asarray(out_arrs[i]).reshape(
                N_CORES, *out_avals[i].shape
            )[c]
            for i, name in enumerate(ex["out_names"])
        }
        for c in range(N_CORES)
    ]
    return BassKernelResults(
        results=results,
        instructions_and_trace=None,
        profile_json=None,
        exec_time_ns=None,
    )


def kernel(**inputs) -> np.ndarray:
    return _run(**inputs, trace=False)[0]


def run_traced(**inputs):
    return _run(**inputs, trace=True)


# revision 20
# speedup vs baseline: 1.0384x; 1.0073x over previous
"""Grouped MoE MLP (SwiGLU) kernel for Trainium2, 8 NeuronCores.

Strategy (pure expert-parallel):
  Tokens arrive pre-sorted by expert with per-expert counts.  Expert e's
  weights and token block go to core e (one expert per core, no weight
  duplication -- weight bytes shipped to the device are the dominant cost
  for this problem).  Token blocks are zero-padded to a common T_pad so
  all cores run one SPMD program.

  Device program per core (dense SwiGLU MLP over T_pad tokens):
    GEMM1: h1^T[f, t] = sum_h W1[h, f] * x[t, h]     (h on partitions)
    SwiGLU on feature-partitioned tiles
    GEMM2: out[t, o]  = sum_f h[t, f] * W2[f, o]     (f on partitions,
           tokens become the PSUM partition dim so the output lands in
           natural [T, HIDDEN] layout -- no transposes on the way out)

  All device I/O is bf16 (inputs cast on host, output cast back to f32
  on host) to halve the bytes staged over the host<->device link.
"""

import math
from contextlib import ExitStack

import ml_dtypes
import numpy as np

P = 128
HIDDEN = 2048
INTER = 1408
GU = 2 * INTER            # 2816 = gate+up columns
KH = HIDDEN // P          # 16 k-tiles for GEMM1
KI = INTER // P           # 11 k-tiles for GEMM2 / gate-up pair blocks
NO = HIDDEN // 512        # 4 output column blocks of 512
N_CORES = 8
NT = 512                  # tokens per chunk (matmul moving free dim)

BF16 = ml_dtypes.bfloat16

_PROGRAM_CACHE: dict = {}


def _build_program(t_pad: int, nt: int):
    import concourse.mybir as mybir
    import concourse.tile as tile
    from concourse import bacc

    bf16 = mybir.dt.bfloat16
    f32 = mybir.dt.float32

    n_chunks = t_pad // nt
    nb = nt // P

    nc = bacc.Bacc(None, target_bir_lowering=False, debug=False)
    # chunk-major layouts: every DMA below moves one fully contiguous
    # DRAM block (x loads 128 KiB, w1 704 KiB, w2 512 KiB, out stores
    # 128 KiB) -- friendly to any memory system the tensors live in.
    # partition-major chunks: each chunk loads as ONE contiguous 2 MiB DMA
    xT = nc.dram_tensor("xT", [n_chunks, P, KH, nt], bf16, kind="ExternalInput")
    # w1 is column-group-major: group g < KI is gate cols [128g, 128g+128),
    # group KI+g is the matching up block -- so GEMM1 group mp only waits
    # for its own two 720 KiB blocks, not the whole 11.5 MiB tensor.
    w1 = nc.dram_tensor("w1", [2 * KI, P, KH, P], bf16, kind="ExternalInput")
    w2 = nc.dram_tensor("w2", [KI, P, HIDDEN], bf16, kind="ExternalInput")
    out = nc.dram_tensor(
        "out", [n_chunks, nb, NO, P, 512], bf16, kind="ExternalOutput"
    )

    with tile.TileContext(nc) as tc, ExitStack() as ctx:
        w1_pool = ctx.enter_context(tc.tile_pool(name="w1p", bufs=1))
        w2_pool = ctx.enter_context(tc.tile_pool(name="w2p", bufs=1))
        x_pool = ctx.enter_context(tc.tile_pool(name="xp", bufs=2))
        h_pool = ctx.enter_context(tc.tile_pool(name="hp", bufs=2))
        g_pool = ctx.enter_context(tc.tile_pool(name="gp", bufs=2))
        o_pool = ctx.enter_context(tc.tile_pool(name="op", bufs=3))
        ps1 = ctx.enter_context(tc.tile_pool(name="ps1", bufs=2, space="PSUM"))
        ps2 = ctx.enter_context(tc.tile_pool(name="ps2", bufs=2, space="PSUM"))

        # first x chunk goes ahead of the weight DMAs so the first GEMM1
        # matmul only waits for w1, not the whole weight set (HWDGE
        # queues are FIFO)
        xt0 = x_pool.tile([P, KH, nt], bf16, tag="xt")
        nc.sync.dma_start(xt0[:], xT[0])
        w1t = w1_pool.tile([P, 2 * KI, KH, P], bf16)
        # interleave gate/up group DMAs in mp order so group mp's two
        # blocks land just before its matmuls need them
        for mp in range(KI):
            nc.sync.dma_start(w1t[:, mp], w1[mp])
            nc.sync.dma_start(w1t[:, KI + mp], w1[KI + mp])
        w2t = w2_pool.tile([P, KI, HIDDEN], bf16)
        for k in range(KI):
            nc.sync.dma_start(w2t[:, k, :], w2[k])

        c_n = nt
        for ci in range(n_chunks):
            if ci == 0:
                xt = xt0
            else:
                xt = x_pool.tile([P, KH, c_n], bf16, tag="xt")
                nc.sync.dma_start(xt[:], xT[ci])
            ht = h_pool.tile([P, KI, c_n], bf16, tag="ht")
            for mp in range(KI):
                pg = ps1.tile([P, c_n], f32, tag="pg")
                pu = ps1.tile([P, c_n], f32, tag="pu")
                for k in range(KH):
                    nc.tensor.matmul(
                        pg[:],
                        w1t[:, mp, k, :],
                        xt[:, k, :],
                        start=(k == 0),
                        stop=(k == KH - 1),
                    )
                for k in range(KH):
                    nc.tensor.matmul(
                        pu[:],
                        w1t[:, KI + mp, k, :],
                        xt[:, k, :],
                        start=(k == 0),
                        stop=(k == KH - 1),
                    )
                gt = g_pool.tile([P, c_n], bf16, tag="gt")
                nc.scalar.activation(
                    gt[:], pg[:], mybir.ActivationFunctionType.Silu
                )
                nc.vector.tensor_mul(ht[:, mp, :], gt[:], pu[:])
            # GEMM2 with tokens on the PSUM partition dim: for each
            # 128-token block, out[tb, o_blk] = ht[:, :, tb].T @ w2
            for tb in range(c_n // P):
                for m in range(NO):
                    po = ps2.tile([P, 512], f32, tag="po")
                    for k in range(KI):
                        nc.tensor.matmul(
                            po[:],
                            ht[:, k, tb * P : (tb + 1) * P],
                            w2t[:, k, m * 512 : (m + 1) * 512],
                            start=(k == 0),
                            stop=(k == KI - 1),
                        )
                    om = o_pool.tile([P, 512], bf16, tag="om")
                    nc.vector.tensor_copy(om[:], po[:])
                    nc.sync.dma_start(out[ci, tb, m], om[:])
    nc.compile()
    return nc


def _get_program(t_pad: int, nt: int):
    key = (t_pad, nt)
    if key not in _PROGRAM_CACHE:
        _PROGRAM_CACHE[key] = _build_program(t_pad, nt)
    return _PROGRAM_CACHE[key]


def _pack_w1(w: np.ndarray) -> np.ndarray:
    # [HIDDEN, GU] f32 -> column-group-major [2*KI, P, KH, 128] bf16
    # (row h = 128k + p, col c = 128g + j)
    return w.reshape(KH, P, 2 * KI, P).transpose(2, 1, 0, 3).astype(BF16)


def _pack_w2(w: np.ndarray) -> np.ndarray:
    # [INTER, HIDDEN] f32 -> [KI, P, HIDDEN] bf16 (pure reshape + cast)
    return w.reshape(KI, P, HIDDEN).astype(BF16)


def _assign_experts(counts):
    """Expert e -> core (e % N_CORES); slots stack when E > N_CORES."""
    n_exp = len(counts)
    n_slots = max(1, math.ceil(n_exp / N_CORES))
    cores = []
    for r in range(N_CORES):
        slots = []
        for s in range(n_slots):
            e = s * N_CORES + r
            slots.append(e if e < n_exp else None)
        cores.append(slots)
    return cores, n_slots


def _run(
    hidden_states: np.ndarray,
    merged_gate_up_proj: np.ndarray,
    merged_down_proj: np.ndarray,
    num_tokens_per_expert: np.ndarray,
    trace: bool = False,
):
    counts = [int(c) for c in np.asarray(num_tokens_per_expert)]
    n_exp = len(counts)
    offs = np.concatenate([[0], np.cumsum(counts)]).astype(int)
    total = int(offs[-1])

    core_experts, n_slots = _assign_experts(counts)
    assert n_slots == 1, "more experts than cores not supported"
    per_core_rows = [sum(counts[e] for e in slots if e is not None)
                     for slots in core_experts]
    t_pad = max(NT, ((max(per_core_rows) + NT - 1) // NT) * NT)

    nc = _get_program(t_pad, NT)

    from concurrent.futures import ThreadPoolExecutor

    pool = ThreadPoolExecutor(8)

    # [TOTAL, HIDDEN] f32 -> bf16 -> transposed [HIDDEN, TOTAL] -> [KH, P, TOTAL]
    x_bf16 = hidden_states[:total].astype(BF16)
    xT_full = np.empty((HIDDEN, total), dtype=BF16)

    def _tr(k):
        xT_full[k * P : (k + 1) * P] = x_bf16[:, k * P : (k + 1) * P].T

    list(pool.map(_tr, range(KH)))
    xT_full = xT_full.reshape(KH, P, total)

    w1_packed = list(pool.map(
        lambda e: _pack_w1(merged_gate_up_proj[e]), range(n_exp)
    ))
    w2_packed = list(pool.map(
        lambda e: _pack_w2(merged_down_proj[e]), range(n_exp)
    ))

    n_chunks = t_pad // NT
    xT_pkt = xT_full.transpose(1, 0, 2)  # [P, KH, total] view

    def _core_x(r):
        e = core_experts[r][0]
        xT_core = np.zeros((n_chunks, P, KH, NT), dtype=BF16)
        if e is not None and counts[e]:
            cnt = counts[e]
            for ci in range(n_chunks):
                t0 = ci * NT
                n = min(NT, cnt - t0)
                if n <= 0:
                    break
                xT_core[ci, :, :, :n] = xT_pkt[
                    :, :, offs[e] + t0 : offs[e] + t0 + n
                ]
        return xT_core

    core_x = list(pool.map(_core_x, range(N_CORES)))
    pool.shutdown(wait=True)

    in_maps = []
    for r in range(N_CORES):
        e = core_experts[r][0]
        ew = e if (e is not None and e < n_exp) else 0
        in_maps.append(
            {"xT": core_x[r], "w1": w1_packed[ew], "w2": w2_packed[ew]}
        )

    res = _execute(nc, in_maps, trace)

    out = np.empty((total, HIDDEN), dtype=np.float32)

    def _unshard(r):
        e = core_experts[r][0]
        if e is None or counts[e] == 0:
            return
        cnt = counts[e]
        # [n_chunks, nb, NO, P, 512] -> [t, o] with t = (ci, tb, p),
        # o = (m, j)
        o_core = np.ascontiguousarray(
            res.results[r]["out"].transpose(0, 1, 3, 2, 4)
        ).reshape(t_pad, HIDDEN)
        out[offs[e] : offs[e] + cnt] = o_core[:cnt].astype(np.float32)

    upool = ThreadPoolExecutor(8)
    list(upool.map(_unshard, range(N_CORES)))
    upool.shutdown(wait=True)
    return out, res


def _execute(nc, in_maps, trace):
    from concourse.bass_utils import run_bass_kernel_spmd

    if not trace:
        try:
            return _execute_pjrt_dev_zeros(nc, in_maps)
        except Exception:
            pass
    # "out" and "xT" have identical byte sizes; the kernel's semaphore
    # chain guarantees every x row is consumed before the corresponding
    # out row is stored, so donating xT's device buffer to out is safe.
    # (On the axon path aliases are ignored; on the native path this
    # skips staging a zero buffer for the output.)
    return run_bass_kernel_spmd(
        nc, in_maps, list(range(N_CORES)), aliases={"out": "xT"}, trace=trace
    )


_EXEC_CACHE: dict = {}


def _build_pjrt_executor(nc):
    from concourse.bass_utils import axon_active
    import concourse.mybir as mybir
    from concourse import bass2jax
    import jax
    import jax.numpy as jnp
    from jax.sharding import Mesh, PartitionSpec, NamedSharding
    from jax.experimental.shard_map import shard_map

    if not axon_active():
        raise RuntimeError("pjrt path requires axon")
    if nc.dbg_addr is not None:
        raise RuntimeError("debug program")

    bass2jax.install_neuronx_cc_hook()

    partition_name = nc.partition_id_tensor.name if nc.partition_id_tensor else None
    in_names, out_names, out_avals = [], [], []
    for alloc in nc.m.functions[0].allocations:
        if not isinstance(alloc, mybir.MemoryLocationSet):
            continue
        name = alloc.memorylocations[0].name
        if alloc.kind == "ExternalInput":
            if name != partition_name:
                in_names.append(name)
        elif alloc.kind == "ExternalOutput":
            out_names.append(name)
            out_avals.append(
                jax.core.ShapedArray(
                    tuple(alloc.tensor_shape), mybir.dt.np(alloc.dtype)
                )
            )
    n_params = len(in_names)
    n_outs = len(out_avals)
    all_names = in_names + out_names
    if partition_name is not None:
        all_names = all_names + [partition_name]
    donate = tuple(range(n_params, n_params + n_outs))

    def _body(*args):
        operands = list(args)
        if partition_name is not None:
            operands.append(bass2jax.partition_id_tensor())
        outs = bass2jax._bass_exec_p.bind(
            *operands,
            out_avals=tuple(out_avals),
            in_names=tuple(all_names),
            out_names=tuple(out_names),
            lowering_input_output_aliases=(),
            sim_require_finite=True,
            sim_require_nnan=True,
            nc=nc,
        )
        return tuple(outs)

    devices = jax.devices()[:N_CORES]
    assert len(devices) == N_CORES
    mesh = Mesh(np.asarray(devices), ("core",))
    in_specs = (PartitionSpec("core"),) * (n_params + n_outs)
    out_specs = (PartitionSpec("core"),) * n_outs
    sharded = jax.jit(
        shard_map(
            _body, mesh=mesh, in_specs=in_specs, out_specs=out_specs,
            check_rep=False,
        ),
        donate_argnums=donate,
        keep_unused=True,
    )
    zsharding = NamedSharding(mesh, PartitionSpec("core"))
    zero_fns = [
        jax.jit(
            lambda s=av.shape, d=av.dtype: jnp.zeros(
                (N_CORES * s[0], *s[1:]), d
            ),
            out_shardings=zsharding,
        )
        for av in out_avals
    ]
    return {
        "sharded": sharded,
        "zero_fns": zero_fns,
        "in_names": in_names,
        "out_names": out_names,
        "out_avals": out_avals,
    }


def _execute_pjrt_dev_zeros(nc, in_maps):
    """run_bass_via_pjrt equivalent, but the donated zero output buffers
    are created on-device (jnp.zeros under jit) instead of being staged
    from host numpy -- saves shipping one full output-sized array of
    zeros per core over the host->device link."""
    from concourse.bass_utils import BassKernelResults

    key = id(nc)
    if key not in _EXEC_CACHE:
        _EXEC_CACHE[key] = _build_pjrt_executor(nc)
    ex = _EXEC_CACHE[key]

    concat_in = [
        np.concatenate([np.asarray(m[name]) for m in in_maps], axis=0)
        for name in ex["in_names"]
    ]
    dev_zeros = [fn() for fn in ex["zero_fns"]]
    out_arrs = ex["sharded"](*concat_in, *dev_zeros)
    out_avals = ex["out_avals"]
    results = [
        {
            name: np.asarray(out_arrs[i]).reshape(
                N_CORES, *out_avals[i].shape
            )[c]
            for i, name in enumerate(ex["out_names"])
        }
        for c in range(N_CORES)
    ]
    return BassKernelResults(
        results=results,
        instructions_and_trace=None,
        profile_json=None,
        exec_time_ns=None,
    )


def kernel(**inputs) -> np.ndarray:
    return _run(**inputs, trace=False)[0]


def run_traced(**inputs):
    return _run(**inputs, trace=True)


# revision 21
# speedup vs baseline: 1.0401x; 1.0016x over previous
"""Grouped MoE MLP (SwiGLU) kernel for Trainium2, 8 NeuronCores.

Strategy (pure expert-parallel):
  Tokens arrive pre-sorted by expert with per-expert counts.  Expert e's
  weights and token block go to core e (one expert per core, no weight
  duplication -- weight bytes shipped to the device are the dominant cost
  for this problem).  Token blocks are zero-padded to a common T_pad so
  all cores run one SPMD program.

  Device program per core (dense SwiGLU MLP over T_pad tokens):
    GEMM1: h1^T[f, t] = sum_h W1[h, f] * x[t, h]     (h on partitions)
    SwiGLU on feature-partitioned tiles
    GEMM2: out[t, o]  = sum_f h[t, f] * W2[f, o]     (f on partitions,
           tokens become the PSUM partition dim so the output lands in
           natural [T, HIDDEN] layout -- no transposes on the way out)

  All device I/O is bf16 (inputs cast on host, output cast back to f32
  on host) to halve the bytes staged over the host<->device link.
"""

import math
from contextlib import ExitStack

import ml_dtypes
import numpy as np

P = 128
HIDDEN = 2048
INTER = 1408
GU = 2 * INTER            # 2816 = gate+up columns
KH = HIDDEN // P          # 16 k-tiles for GEMM1
KI = INTER // P           # 11 k-tiles for GEMM2 / gate-up pair blocks
NO = HIDDEN // 512        # 4 output column blocks of 512
N_CORES = 8
NT = 512                  # tokens per chunk (matmul moving free dim)

BF16 = ml_dtypes.bfloat16

_PROGRAM_CACHE: dict = {}


def _build_program(t_pad: int, nt: int):
    import concourse.mybir as mybir
    import concourse.tile as tile
    from concourse import bacc

    bf16 = mybir.dt.bfloat16
    f32 = mybir.dt.float32

    n_chunks = t_pad // nt
    nb = nt // P

    nc = bacc.Bacc(None, target_bir_lowering=False, debug=False)
    # chunk-major layouts: every DMA below moves one fully contiguous
    # DRAM block (x loads 128 KiB, w1 704 KiB, w2 512 KiB, out stores
    # 128 KiB) -- friendly to any memory system the tensors live in.
    # partition-major chunks: each chunk loads as ONE contiguous 2 MiB DMA
    xT = nc.dram_tensor("xT", [n_chunks, P, KH, nt], bf16, kind="ExternalInput")
    # w1 is column-group-major: group g < KI is gate cols [128g, 128g+128),
    # group KI+g is the matching up block -- so GEMM1 group mp only waits
    # for its own two 720 KiB blocks, not the whole 11.5 MiB tensor.
    w1 = nc.dram_tensor("w1", [2 * KI, P, KH, P], bf16, kind="ExternalInput")
    w2 = nc.dram_tensor("w2", [KI, P, HIDDEN], bf16, kind="ExternalInput")
    out = nc.dram_tensor(
        "out", [n_chunks, nb, NO, P, 512], bf16, kind="ExternalOutput"
    )

    with tile.TileContext(nc) as tc, ExitStack() as ctx:
        w1_pool = ctx.enter_context(tc.tile_pool(name="w1p", bufs=1))
        w2_pool = ctx.enter_context(tc.tile_pool(name="w2p", bufs=1))
        x_pool = ctx.enter_context(tc.tile_pool(name="xp", bufs=2))
        h_pool = ctx.enter_context(tc.tile_pool(name="hp", bufs=2))
        g_pool = ctx.enter_context(tc.tile_pool(name="gp", bufs=2))
        o_pool = ctx.enter_context(tc.tile_pool(name="op", bufs=4))
        ps1 = ctx.enter_context(tc.tile_pool(name="ps1", bufs=2, space="PSUM"))
        ps2 = ctx.enter_context(tc.tile_pool(name="ps2", bufs=4, space="PSUM"))

        # first x chunk goes ahead of the weight DMAs so the first GEMM1
        # matmul only waits for w1, not the whole weight set (HWDGE
        # queues are FIFO)
        xt0 = x_pool.tile([P, KH, nt], bf16, tag="xt")
        nc.sync.dma_start(xt0[:], xT[0])
        w1t = w1_pool.tile([P, 2 * KI, KH, P], bf16)
        # interleave gate/up group DMAs in mp order so group mp's two
        # blocks land just before its matmuls need them
        for mp in range(KI):
            nc.sync.dma_start(w1t[:, mp], w1[mp])
            nc.sync.dma_start(w1t[:, KI + mp], w1[KI + mp])
        w2t = w2_pool.tile([P, KI, HIDDEN], bf16)
        for k in range(KI):
            nc.sync.dma_start(w2t[:, k, :], w2[k])

        c_n = nt
        for ci in range(n_chunks):
            if ci == 0:
                xt = xt0
            else:
                xt = x_pool.tile([P, KH, c_n], bf16, tag="xt")
                nc.sync.dma_start(xt[:], xT[ci])
            ht = h_pool.tile([P, KI, c_n], bf16, tag="ht")
            for mp in range(KI):
                pg = ps1.tile([P, c_n], f32, tag="pg")
                pu = ps1.tile([P, c_n], f32, tag="pu")
                for k in range(KH):
                    nc.tensor.matmul(
                        pg[:],
                        w1t[:, mp, k, :],
                        xt[:, k, :],
                        start=(k == 0),
                        stop=(k == KH - 1),
                    )
                for k in range(KH):
                    nc.tensor.matmul(
                        pu[:],
                        w1t[:, KI + mp, k, :],
                        xt[:, k, :],
                        start=(k == 0),
                        stop=(k == KH - 1),
                    )
                gt = g_pool.tile([P, c_n], bf16, tag="gt")
                nc.scalar.activation(
                    gt[:], pg[:], mybir.ActivationFunctionType.Silu
                )
                nc.vector.tensor_mul(ht[:, mp, :], gt[:], pu[:])
            # GEMM2 with tokens on the PSUM partition dim: for each
            # 128-token block, out[tb, o_blk] = ht[:, :, tb].T @ w2
            for tb in range(c_n // P):
                for m in range(NO):
                    po = ps2.tile([P, 512], f32, tag="po")
                    for k in range(KI):
                        nc.tensor.matmul(
                            po[:],
                            ht[:, k, tb * P : (tb + 1) * P],
                            w2t[:, k, m * 512 : (m + 1) * 512],
                            start=(k == 0),
                            stop=(k == KI - 1),
                        )
                    om = o_pool.tile([P, 512], bf16, tag="om")
                    nc.vector.tensor_copy(om[:], po[:])
                    nc.sync.dma_start(out[ci, tb, m], om[:])
    nc.compile()
    return nc


def _get_program(t_pad: int, nt: int):
    key = (t_pad, nt)
    if key not in _PROGRAM_CACHE:
        _PROGRAM_CACHE[key] = _build_program(t_pad, nt)
    return _PROGRAM_CACHE[key]


def _pack_w1(w: np.ndarray) -> np.ndarray:
    # [HIDDEN, GU] f32 -> column-group-major [2*KI, P, KH, 128] bf16
    # (row h = 128k + p, col c = 128g + j)
    return w.reshape(KH, P, 2 * KI, P).transpose(2, 1, 0, 3).astype(BF16)


def _pack_w2(w: np.ndarray) -> np.ndarray:
    # [INTER, HIDDEN] f32 -> [KI, P, HIDDEN] bf16 (pure reshape + cast)
    return w.reshape(KI, P, HIDDEN).astype(BF16)


def _assign_experts(counts):
    """Expert e -> core (e % N_CORES); slots stack when E > N_CORES."""
    n_exp = len(counts)
    n_slots = max(1, math.ceil(n_exp / N_CORES))
    cores = []
    for r in range(N_CORES):
        slots = []
        for s in range(n_slots):
            e = s * N_CORES + r
            slots.append(e if e < n_exp else None)
        cores.append(slots)
    return cores, n_slots


def _run(
    hidden_states: np.ndarray,
    merged_gate_up_proj: np.ndarray,
    merged_down_proj: np.ndarray,
    num_tokens_per_expert: np.ndarray,
    trace: bool = False,
):
    counts = [int(c) for c in np.asarray(num_tokens_per_expert)]
    n_exp = len(counts)
    offs = np.concatenate([[0], np.cumsum(counts)]).astype(int)
    total = int(offs[-1])

    core_experts, n_slots = _assign_experts(counts)
    assert n_slots == 1, "more experts than cores not supported"
    per_core_rows = [sum(counts[e] for e in slots if e is not None)
                     for slots in core_experts]
    t_pad = max(NT, ((max(per_core_rows) + NT - 1) // NT) * NT)

    nc = _get_program(t_pad, NT)

    from concurrent.futures import ThreadPoolExecutor

    pool = ThreadPoolExecutor(8)

    # [TOTAL, HIDDEN] f32 -> bf16 -> transposed [HIDDEN, TOTAL] -> [KH, P, TOTAL]
    x_bf16 = hidden_states[:total].astype(BF16)
    xT_full = np.empty((HIDDEN, total), dtype=BF16)

    def _tr(k):
        xT_full[k * P : (k + 1) * P] = x_bf16[:, k * P : (k + 1) * P].T

    list(pool.map(_tr, range(KH)))
    xT_full = xT_full.reshape(KH, P, total)

    w1_packed = list(pool.map(
        lambda e: _pack_w1(merged_gate_up_proj[e]), range(n_exp)
    ))
    w2_packed = list(pool.map(
        lambda e: _pack_w2(merged_down_proj[e]), range(n_exp)
    ))

    n_chunks = t_pad // NT
    xT_pkt = xT_full.transpose(1, 0, 2)  # [P, KH, total] view

    def _core_x(r):
        e = core_experts[r][0]
        xT_core = np.zeros((n_chunks, P, KH, NT), dtype=BF16)
        if e is not None and counts[e]:
            cnt = counts[e]
            for ci in range(n_chunks):
                t0 = ci * NT
                n = min(NT, cnt - t0)
                if n <= 0:
                    break
                xT_core[ci, :, :, :n] = xT_pkt[
                    :, :, offs[e] + t0 : offs[e] + t0 + n
                ]
        return xT_core

    core_x = list(pool.map(_core_x, range(N_CORES)))
    pool.shutdown(wait=True)

    in_maps = []
    for r in range(N_CORES):
        e = core_experts[r][0]
        ew = e if (e is not None and e < n_exp) else 0
        in_maps.append(
            {"xT": core_x[r], "w1": w1_packed[ew], "w2": w2_packed[ew]}
        )

    res = _execute(nc, in_maps, trace)

    out = np.empty((total, HIDDEN), dtype=np.float32)

    def _unshard(r):
        e = core_experts[r][0]
        if e is None or counts[e] == 0:
            return
        cnt = counts[e]
        # [n_chunks, nb, NO, P, 512] -> [t, o] with t = (ci, tb, p),
        # o = (m, j)
        o_core = np.ascontiguousarray(
            res.results[r]["out"].transpose(0, 1, 3, 2, 4)
        ).reshape(t_pad, HIDDEN)
        out[offs[e] : offs[e] + cnt] = o_core[:cnt].astype(np.float32)

    upool = ThreadPoolExecutor(8)
    list(upool.map(_unshard, range(N_CORES)))
    upool.shutdown(wait=True)
    return out, res


def _execute(nc, in_maps, trace):
    from concourse.bass_utils import run_bass_kernel_spmd

    if not trace:
        try:
            return _execute_pjrt_dev_zeros(nc, in_maps)
        except Exception:
            pass
    # "out" and "xT" have identical byte sizes; the kernel's semaphore
    # chain guarantees every x row is consumed before the corresponding
    # out row is stored, so donating xT's device buffer to out is safe.
    # (On the axon path aliases are ignored; on the native path this
    # skips staging a zero buffer for the output.)
    return run_bass_kernel_spmd(
        nc, in_maps, list(range(N_CORES)), aliases={"out": "xT"}, trace=trace
    )


_EXEC_CACHE: dict = {}


def _build_pjrt_executor(nc):
    from concourse.bass_utils import axon_active
    import concourse.mybir as mybir
    from concourse import bass2jax
    import jax
    import jax.numpy as jnp
    from jax.sharding import Mesh, PartitionSpec, NamedSharding
    from jax.experimental.shard_map import shard_map

    if not axon_active():
        raise RuntimeError("pjrt path requires axon")
    if nc.dbg_addr is not None:
        raise RuntimeError("debug program")

    bass2jax.install_neuronx_cc_hook()

    partition_name = nc.partition_id_tensor.name if nc.partition_id_tensor else None
    in_names, out_names, out_avals = [], [], []
    for alloc in nc.m.functions[0].allocations:
        if not isinstance(alloc, mybir.MemoryLocationSet):
            continue
        name = alloc.memorylocations[0].name
        if alloc.kind == "ExternalInput":
            if name != partition_name:
                in_names.append(name)
        elif alloc.kind == "ExternalOutput":
            out_names.append(name)
            out_avals.append(
                jax.core.ShapedArray(
                    tuple(alloc.tensor_shape), mybir.dt.np(alloc.dtype)
                )
            )
    n_params = len(in_names)
    n_outs = len(out_avals)
    all_names = in_names + out_names
    if partition_name is not None:
        all_names = all_names + [partition_name]
    donate = tuple(range(n_params, n_params + n_outs))

    def _body(*args):
        operands = list(args)
        if partition_name is not None:
            operands.append(bass2jax.partition_id_tensor())
        outs = bass2jax._bass_exec_p.bind(
            *operands,
            out_avals=tuple(out_avals),
            in_names=tuple(all_names),
            out_names=tuple(out_names),
            lowering_input_output_aliases=(),
            sim_require_finite=True,
            sim_require_nnan=True,
            nc=nc,
        )
        return tuple(outs)

    devices = jax.devices()[:N_CORES]
    assert len(devices) == N_CORES
    mesh = Mesh(np.asarray(devices), ("core",))
    in_specs = (PartitionSpec("core"),) * (n_params + n_outs)
    out_specs = (PartitionSpec("core"),) * n_outs
    sharded = jax.jit(
        shard_map(
            _body, mesh=mesh, in_specs=in_specs, out_specs=out_specs,
            check_rep=False,
        ),
        donate_argnums=donate,
        keep_unused=True,
    )
    zsharding = NamedSharding(mesh, PartitionSpec("core"))
    zero_fns = [
        jax.jit(
            lambda s=av.shape, d=av.dtype: jnp.zeros(
                (N_CORES * s[0], *s[1:]), d
            ),
            out_shardings=zsharding,
        )
        for av in out_avals
    ]
    return {
        "sharded": sharded,
        "zero_fns": zero_fns,
        "in_names": in_names,
        "out_names": out_names,
        "out_avals": out_avals,
    }


def _execute_pjrt_dev_zeros(nc, in_maps):
    """run_bass_via_pjrt equivalent, but the donated zero output buffers
    are created on-device (jnp.zeros under jit) instead of being staged
    from host numpy -- saves shipping one full output-sized array of
    zeros per core over the host->device link."""
    from concourse.bass_utils import BassKernelResults

    key = id(nc)
    if key not in _EXEC_CACHE:
        _EXEC_CACHE[key] = _build_pjrt_executor(nc)
    ex = _EXEC_CACHE[key]

    concat_in = [
        np.concatenate([np.asarray(m[name]) for m in in_maps], axis=0)
        for name in ex["in_names"]
    ]
    dev_zeros = [fn() for fn in ex["zero_fns"]]
    out_arrs = ex["sharded"](*concat_in, *dev_zeros)
    out_avals = ex["out_avals"]
    results = [
        {
            name: np.asarray(out_arrs[i]).reshape(
                N_CORES, *out_avals[i].shape
            )[c]
            for i, name in enumerate(ex["out_names"])
        }
        for c in range(N_CORES)
    ]
    return BassKernelResults(
        results=results,
        instructions_and_trace=None,
        profile_json=None,
        exec_time_ns=None,
    )


def kernel(**inputs) -> np.ndarray:
    return _run(**inputs, trace=False)[0]


def run_traced(**inputs):
    return _run(**inputs, trace=True)
